# revision 40
# baseline (speedup 1.0000x reference)
"""Trainium2 Bass kernel for nn_AttentionBlock (GroupNorm + MHA + out-proj + residual).

Sharding: pure data-parallel over batch B=16 across 8 NeuronCores (2 per core).

v2 redesign vs the 205us baseline (trace-driven):
  - The kernel is elementwise-bound: exp (131K elem/partition) + PSUM
    evictions can only run on ScalarE/VectorE (~1 elem/ns each). So the
    exp stream and ALL evictions are POOLED across both engines:
    ScalarE does true exp (activation, scale/bias fused) for ~half the
    kt slots, DVE does Schraudolph bit-trick exp for the rest; qk/v/proj
    evictions go to whichever engine the slot pattern assigns.
  - qk eviction+bias on ScalarE via activation(Identity, bias=AP).
  - v bias and out-proj bias via K=1 ones-row matmuls on the PE;
    residual via identity @ x_bf16 matmul accumulated into the proj
    psum. The f32 x copy is gone (halves input DMA); evictions become
    engine-agnostic plain copies.
  - GroupNorm affine apply on GpSimd (SBUF->SBUF, per-partition scalars).
  - PSUM: one shared 3x[128,2,512] pool (scores/qkv/proj rotate through
    it) + 1 bank av + 1 bank den = exactly 8 banks.
  - Prologue: dummy exp at t=0 preloads the ACT exp table; x_bf load
    fans across both HWDGE rings; denser PE warmup keeps HAM warm.
"""
import os
import sys

for _p in ("/opt/trn_rl_repo",):
    if _p not in sys.path and os.path.isdir(_p):
        sys.path.insert(0, _p)

import numpy as np

import concourse.bass as bass
import concourse.bacc as bacc
import concourse.mybir as mybir
import concourse.tile as tile

F32 = mybir.dt.float32
BF16 = mybir.dt.bfloat16
FP8 = mybir.dt.float8e4
I16 = mybir.dt.int16
I32 = mybir.dt.int32
U8 = mybir.dt.uint8
FP8V = mybir.dt.float8e5   # wide-range fp8 for softmax weights / v

B_LOCAL = 2        # batch elements per core
L = 1024           # tokens (H*W)
C = 512            # channels
NH = 8             # heads
D = 64             # head dim
GROUPS = 32
GSIZE = C // GROUPS  # 16
EPS = 1e-5
NCHUNK = C // 128    # 4 channel chunks
NTT = L // 128       # 8 token tiles
SCALE = 1.0 / 8.0    # (1/sqrt(sqrt(64)))**2 applied inside exp
EXP_BIAS = -0.7      # common exp shift; cancels in softmax
# DVE bit-trick exp: bf16 bits of exp(SCALE*s + EXP_BIAS) ~= EXPA*s + EXPB
EXPA = 128.0 * np.log2(np.e) * SCALE
EXPB = 128.0 * (127.0 + EXP_BIAS * np.log2(np.e))
# fp8e5m2 bit-trick variant (4 bits/octave, bias 15), uint8-saturating at 0.
# e5m2's 30-octave range covers the full softmax weight spread (scores hit
# +-68 in this data; e4m3 overflows at exp>448).
EXPA8 = 4.0 * np.log2(np.e) * SCALE
EXPB8 = 4.0 * (15.0 + EXP_BIAS * np.log2(np.e))

# Scalar-engine exp slots per unit (rest go to DVE Schraudolph).
SEXP_KTS = (0, 1, 4, 6)


def build_attention_block(tc, ctx):
    nc = tc.nc
    AF = mybir.ActivationFunctionType
    OP = mybir.AluOpType
    DR = mybir.MatmulPerfMode.DoubleRow

    xbf_d = nc.dram_tensor("x_bf", [B_LOCAL, C, L], BF16, kind="ExternalInput").ap()
    gamma_d = nc.dram_tensor("gamma", [C], F32, kind="ExternalInput").ap()
    beta_d = nc.dram_tensor("beta", [C], F32, kind="ExternalInput").ap()
    wq_d = nc.dram_tensor("w_qkv", [C, 3 * C], FP8, kind="ExternalInput").ap()
    bq_d = nc.dram_tensor("b_qkv", [3 * C], F32, kind="ExternalInput").ap()
    wo_d = nc.dram_tensor("w_out", [C, C], FP8, kind="ExternalInput").ap()
    bvo_d = nc.dram_tensor("bvo_bf", [2, C], BF16, kind="ExternalInput").ap()
    ident_d = nc.dram_tensor("ident", [128, 128], BF16, kind="ExternalInput").ap()
    out_d = nc.dram_tensor("out", [B_LOCAL, C, L], F32, kind="ExternalOutput").ap()

    singles = ctx.enter_context(tc.tile_pool(name="singles", bufs=1))
    big = ctx.enter_context(tc.tile_pool(name="big", bufs=2))
    small = ctx.enter_context(tc.tile_pool(name="small", bufs=3))
    epool = ctx.enter_context(tc.tile_pool(name="epool", bufs=2))
    rpool = ctx.enter_context(tc.tile_pool(name="rpool", bufs=3))
    hpool = ctx.enter_context(tc.tile_pool(name="hpool", bufs=3))
    pbig = ctx.enter_context(tc.tile_pool(name="pbig", bufs=3, space="PSUM"))
    pav = ctx.enter_context(tc.tile_pool(name="pav", bufs=1, space="PSUM"))
    pden = ctx.enter_context(tc.tile_pool(name="pden", bufs=1, space="PSUM"))

    # ---- one-time constants ----
    # warm-up operand first: DVE memset completes in ~200ns so the PE can
    # start its warm-up spin immediately (gpsimd issues far too slowly).
    wtile = singles.tile([128, 256], BF16)
    nc.vector.memset(wtile, 0.001)
    ebias_sb = singles.tile([128, 1], F32)
    nc.gpsimd.memset(ebias_sb, EXP_BIAS)
    # dummy exp: forces the ACT exp-table load at t~0 (hidden under x DMA)
    dummy = singles.tile([128, 1], BF16)
    nc.scalar.activation(dummy, ebias_sb, AF.Exp)


    ones_sb = singles.tile([128, D], BF16)
    nc.gpsimd.memset(ones_sb, 1.0)
    ones_n = singles.tile([1, 512], BF16)
    nc.gpsimd.memset(ones_n, 1.0)

    # e_mat[c, g] = 1 iff c//16 == g (band built via two affine selects)
    e_mat = singles.tile([128, 8], F32)       # channel -> group indicator
    nc.gpsimd.memset(e_mat, 1.0)
    nc.gpsimd.affine_select(out=e_mat, in_=e_mat, compare_op=OP.is_ge,
                            fill=0.0, base=0, pattern=[[-GSIZE, 8]],
                            channel_multiplier=1)
    nc.gpsimd.affine_select(out=e_mat, in_=e_mat, compare_op=OP.is_ge,
                            fill=0.0, base=GSIZE - 1, pattern=[[GSIZE, 8]],
                            channel_multiplier=-1)
    e2_mat = singles.tile([8, 128], F32)      # group -> channel indicator
    nc.gpsimd.memset(e2_mat, 1.0)
    nc.gpsimd.affine_select(out=e2_mat, in_=e2_mat, compare_op=OP.is_ge,
                            fill=0.0, base=0, pattern=[[1, 128]],
                            channel_multiplier=-GSIZE)
    nc.gpsimd.affine_select(out=e2_mat, in_=e2_mat, compare_op=OP.is_ge,
                            fill=0.0, base=GSIZE - 1, pattern=[[-1, 128]],
                            channel_multiplier=GSIZE)

    wq8 = singles.tile([128, NCHUNK, 3 * C], FP8)
    wo8 = singles.tile([128, NCHUNK, C], FP8)
    ident = singles.tile([128, 128], BF16)
    bv_row = singles.tile([1, C], BF16)       # v bias as a K=1 weight row
    bo_row = singles.tile([1, C], BF16)       # out bias as a K=1 weight row
    gamma_sb = singles.tile([128, NCHUNK], F32)
    beta_sb = singles.tile([128, NCHUNK], F32)
    bqk_sb = singles.tile([128, 8], F32)      # q,k biases per [partition, fi]

    def load_weights():
        nc.gpsimd.dma_start(gamma_sb, gamma_d.rearrange("(o p) -> p o", p=128))
        nc.gpsimd.dma_start(beta_sb, beta_d.rearrange("(o p) -> p o", p=128))
        nc.gpsimd.dma_start(bqk_sb, bq_d[0:2 * C].rearrange("(o p) -> p o", p=128))
        nc.gpsimd.dma_start(bv_row, bvo_d[0:1, :])
        nc.gpsimd.dma_start(bo_row, bvo_d[1:2, :])
        nc.gpsimd.dma_start(ident, ident_d)

    def load_wq():
        # weights arrive host-cast to fp8e4; q,k columns first
        # (prologue-critical), then v, then w_out
        wq_r = wq_d.rearrange("(o p) f -> p o f", p=128)
        for kc in range(NCHUNK):
            nc.sync.dma_start(wq8[:, kc, 0:2 * C], wq_r[:, kc, 0:2 * C])
        for kc in range(NCHUNK):
            nc.sync.dma_start(wq8[:, kc, 2 * C:3 * C], wq_r[:, kc, 2 * C:3 * C])
        nc.sync.dma_start(wo8, wo_d.rearrange("(o p) f -> p o f", p=128))

    def load_xbf(b):
        """x^T in bf16 (host-cast). Fan the 8 half-chunk descriptors across
        both HWDGE rings so the prologue-critical load lands fast."""
        xTbf = big.tile([128, NCHUNK, L], BF16, tag="xTbf")
        xT8 = big.tile([128, NCHUNK, L], FP8, tag="xT8")
        for cc in range(NCHUNK):
            c0 = cc * 128
            eng0 = nc.sync if b == 0 else nc.scalar
            eng0.dma_start(xTbf[0:64, cc], xbf_d[b, c0:c0 + 64, :])
            nc.scalar.dma_start(xTbf[64:128, cc], xbf_d[b, c0 + 64:c0 + 128, :])
        return xTbf, xT8

    gn_scratch = singles.tile([128, 1024], BF16)

    def gn_steps(xTp, dve_applies=(0, 1), stats_scalar=False):
        """GroupNorm over all 4 chunks as a list of emission steps (so b1's
        GN can be sprinkled into filler slots). One gs/bc PE round-trip for
        the whole batch element. Applies split across DVE and GpSimd."""
        xTbf, xT8 = xTp
        box = {}

        def stats(cc):
            if cc == 0:
                box["mv"] = small.tile([128, 4, 2], F32, tag="mv", name="mv")
            if stats_scalar:
                # sum and sum-of-squares via ScalarE activation accumulators
                # (keeps b1's GN stats off the busy DVE entirely)
                nc.scalar.activation(gn_scratch, xTbf[:, cc, :], AF.Identity,
                                     accum_out=box["mv"][:, cc, 0:1])
                nc.scalar.activation(gn_scratch, xTbf[:, cc, :], AF.Square,
                                     accum_out=box["mv"][:, cc, 1:2])
                return
            st = small.tile([128, 2, 6], F32, tag="bnst")
            for s in range(2):
                nc.vector.bn_stats(st[:, s], xTbf[:, cc, s * 512:(s + 1) * 512])
            nc.vector.bn_aggr(box["mv"][:, cc, :], st)

        def smalls_gs():
            mv = box["mv"]
            if stats_scalar:
                # mv already holds [sum_c, sumsq_c]; the group matmul sums
                # them and the /(GSIZE*L) scaling happens in smalls_bc
                box["sq"] = mv
            else:
                sq = small.tile([128, 4, 2], F32, tag="sq")  # [m_c, E[x^2]_c]
                nc.vector.tensor_copy(sq[:, :, 0], mv[:, :, 0])
                nc.vector.tensor_tensor(sq[:, :, 1], mv[:, :, 0], mv[:, :, 0],
                                        op=OP.mult)
                nc.vector.tensor_tensor(sq[:, :, 1], sq[:, :, 1], mv[:, :, 1],
                                        op=OP.add)
                box["sq"] = sq
            gs = pbig.tile([8, 8], F32, tag="mm")         # per-group sums
            nc.tensor.matmul(gs, lhsT=e_mat,
                             rhs=box["sq"].rearrange("p a b -> p (a b)"),
                             start=True, stop=True)
            box["gs"] = gs

        def smalls_bc():
            gs = box["gs"]
            gsb = small.tile([8, 4, 2], F32, tag="gsb")
            scl = 1.0 / (GSIZE * L) if stats_scalar else 1.0 / GSIZE
            nc.vector.tensor_scalar_mul(gsb, gs.rearrange("p (a b) -> p a b", b=2),
                                        scl)              # [m_g, E[x^2]_g]
            var = small.tile([8, 4], F32, tag="var")
            nc.vector.tensor_tensor(var, gsb[:, :, 0], gsb[:, :, 0], op=OP.mult)
            nc.vector.tensor_tensor(var, gsb[:, :, 1], var, op=OP.subtract)
            nc.vector.tensor_scalar(out=var, in0=var, scalar1=float(EPS),
                                    scalar2=None, op0=OP.add)
            # rstd = rsqrt(var+eps): Quake-III seed + two Newton steps
            yi = small.tile([8, 4], I32, tag="yi")
            nc.vector.tensor_scalar(out=yi, in0=var.bitcast(I32),
                                    scalar1=1, scalar2=None,
                                    op0=OP.arith_shift_right)
            nc.vector.tensor_scalar(out=yi, in0=yi, scalar1=-1, scalar2=0x5F3759DF,
                                    op0=OP.mult, op1=OP.add)
            y = yi.bitcast(F32)
            t = small.tile([8, 4], F32, tag="nrt")
            for _ in range(2):
                nc.vector.tensor_tensor(t, y, y, op=OP.mult)
                nc.vector.tensor_tensor(t, t, var, op=OP.mult)
                nc.vector.tensor_scalar(out=t, in0=t, scalar1=-0.5, scalar2=1.5,
                                        op0=OP.mult, op1=OP.add)
                nc.vector.tensor_tensor(y, y, t, op=OP.mult)
            nc.vector.tensor_copy(gsb[:, :, 1], y)        # gsb = [m_g, rstd_g]
            bc = pbig.tile([128, 8], F32, tag="mm")       # broadcast via PE
            nc.tensor.matmul(bc, lhsT=e2_mat,
                             rhs=gsb.rearrange("p a b -> p (a b)"),
                             start=True, stop=True)
            bc2 = bc.rearrange("p (a b) -> p a b", b=2)
            ab = small.tile([128, 4, 2], F32, tag="ab")
            nc.vector.tensor_tensor(ab[:, :, 0], bc2[:, :, 1], gamma_sb,
                                    op=OP.mult)
            nc.vector.tensor_tensor(ab[:, :, 1], bc2[:, :, 0], ab[:, :, 0],
                                    op=OP.mult)
            nc.vector.tensor_tensor(ab[:, :, 1], beta_sb, ab[:, :, 1],
                                    op=OP.subtract)
            box["ab"] = ab

        def apply(cc):
            ab = box["ab"]
            eng = nc.vector if cc in dve_applies else nc.gpsimd
            eng.tensor_scalar(out=xT8[:, cc, :], in0=xTbf[:, cc, :],
                              scalar1=ab[:, cc, 0:1], scalar2=ab[:, cc, 1:2],
                              op0=OP.mult, op1=OP.add)

        return [lambda: stats(0), lambda: stats(1), lambda: stats(2),
                lambda: stats(3), smalls_gs, smalls_bc,
                lambda: apply(0), lambda: apply(1), lambda: apply(2),
                lambda: apply(3)]

    def alloc_qkv():
        qkT = big.tile([128, 8, L], BF16, tag="qkT")
        v_sb = big.tile([128, NTT, NH, D], BF16, tag="v")
        return qkT, v_sb

    def stage_qk(xT8, qkT, fi, eng_s):
        """One q/k feature tile fi (128 features x full L) : 4 DR matmuls into
        a [128,2,512] psum, one pooled evict+bias."""
        ps = pbig.tile([128, 2, 512], F32, tag="mm")
        for tb in range(2):
            for g in range(2):
                nc.tensor.matmul(
                    ps[:, tb],
                    lhsT=wq8[:, 2 * g:2 * g + 2, fi * 128:(fi + 1) * 128],
                    rhs=xT8[:, 2 * g:2 * g + 2, tb * 512:(tb + 1) * 512],
                    start=(g == 0), stop=(g == 1), perf_mode=DR,
                )
        if eng_s:
            nc.scalar.activation(qkT[:, fi, :].rearrange("p (a b) -> p a b", a=2),
                                 ps, AF.Identity, bias=bqk_sb[:, fi:fi + 1],
                                 scale=1.0)
        else:
            nc.vector.tensor_scalar(
                out=qkT[:, fi, :].rearrange("p (a b) -> p a b", a=2), in0=ps,
                scalar1=bqk_sb[:, fi:fi + 1], scalar2=None, op0=OP.add)

    def stage_v(xT8, v_sb, tp, eng_s):
        """v for token-tile pair (2tp, 2tp+1): fp8 DR matmuls + K=1 ones-row
        bias matmul, then an engine-agnostic copy eviction."""
        ps = pbig.tile([128, 2, 512], F32, tag="mm")
        for half in range(2):
            tt = 2 * tp + half
            for g in range(2):
                nc.tensor.matmul(
                    ps[:, half],
                    lhsT=xT8[:, 2 * g:2 * g + 2, tt * 128:(tt + 1) * 128],
                    rhs=wq8[:, 2 * g:2 * g + 2, 2 * C:3 * C],
                    start=(g == 0), stop=False, perf_mode=DR,
                )
            nc.tensor.matmul(ps[:, half], lhsT=ones_n[:, 0:128], rhs=bv_row,
                             start=False, stop=True)
        dst = v_sb[:, 2 * tp:2 * tp + 2].rearrange("p a h d -> p a (h d)")
        if eng_s:
            nc.scalar.activation(dst, ps, AF.Copy)
        else:
            nc.vector.tensor_copy(dst, ps)

    class UnitEmitter:
        """Attention for head pair (2*hp, 2*hp+1), query half qb.
        Per k-tile: row-packed score pair -> one exp on ScalarE (true exp)
        or DVE (Schraudolph bit-trick) -> col-packed attn@v pair + den."""

        def __init__(self, qkT, v_sb, aT, b, hp, qb, sexp_kts=SEXP_KTS):
            self.qkT, self.v_sb, self.aT = qkT, v_sb, aT
            self.b, self.hp, self.qb = b, hp, qb
            self.qs = slice(qb * 512, (qb + 1) * 512)
            self.sexp_kts = sexp_kts
            self.ready = False

        def sc_exp(self, kt):
            if not self.ready:
                self.ebf = epool.tile([128, NTT, 2, 512], BF16, tag="e")
                self.ready = True
            hp, qs = self.hp, self.qs
            ks = slice(kt * 128, (kt + 1) * 128)
            sc = pbig.tile([128, 2, 512], F32, tag="mm")
            nc.tensor.matmul(sc[:, 0], lhsT=self.qkT[0:64, 4 + hp, ks],
                             rhs=self.qkT[0:64, hp, qs], start=True, stop=True)
            nc.tensor.matmul(sc[:, 1], lhsT=self.qkT[64:128, 4 + hp, ks],
                             rhs=self.qkT[64:128, hp, qs], start=True, stop=True)
            if kt in self.sexp_kts:
                nc.scalar.activation(self.ebf[:, kt], sc, AF.Exp, bias=ebias_sb,
                                     scale=SCALE)
            else:
                # Schraudolph exp on DVE: write the bf16 bit pattern of
                # exp(SCALE*s + EXP_BIAS) via one int16 linear op
                nc.vector.tensor_scalar(
                    out=self.ebf[:, kt].bitcast(I16), in0=sc,
                    scalar1=float(EXPA), scalar2=float(EXPB),
                    op0=OP.mult, op1=OP.add)

        def av_den(self, g):
            if g == 0:
                self.out_p = pav.tile([128, 512], F32, tag="aout")
                self.den_p = pden.tile([128, 512], F32, tag="aden")
            h0, h1 = 2 * self.hp, 2 * self.hp + 1
            for kt in (2 * g, 2 * g + 1):
                nc.tensor.matmul(self.out_p[0:64, :], lhsT=self.v_sb[:, kt, h0],
                                 rhs=self.ebf[:, kt, 0], start=(kt == 0),
                                 stop=(kt == NTT - 1), skip_group_check=True)
                nc.tensor.matmul(self.out_p[64:128, :], lhsT=self.v_sb[:, kt, h1],
                                 rhs=self.ebf[:, kt, 1], start=(kt == 0),
                                 stop=(kt == NTT - 1), skip_group_check=True)
                nc.tensor.matmul(self.den_p[0:64, :], lhsT=ones_sb,
                                 rhs=self.ebf[:, kt, 0], start=(kt == 0),
                                 stop=(kt == NTT - 1), skip_group_check=True)
                nc.tensor.matmul(self.den_p[64:128, :], lhsT=ones_sb,
                                 rhs=self.ebf[:, kt, 1], start=(kt == 0),
                                 stop=(kt == NTT - 1), skip_group_check=True)

        def finalize_a(self):
            rc = rpool.tile([128, 512], F32, tag="rc")
            nc.vector.reciprocal_approx_fast(rc, self.den_p)
            nc.vector.tensor_tensor(out=self.aT[:, self.hp, self.qs],
                                    in0=self.out_p, in1=rc, op=OP.mult)

        def finalize_b(self):
            pass

    def proj_cp(b, aT, xTbf, cp, th, eng_s, tail=False):
        """Transposed out-projection for channel chunks (2cp, 2cp+1), token
        half th: fp8 DR matmuls + identity@x residual + K=1 b_out row,
        then a pooled copy eviction and the store."""
        ts = slice(th * 512, (th + 1) * 512)
        ps = pbig.tile([128, 2, 512], F32, tag="mm")
        for j in range(2):
            co = 2 * cp + j
            for g in range(2):
                nc.tensor.matmul(
                    ps[:, j],
                    lhsT=wo8[:, 2 * g:2 * g + 2, co * 128:(co + 1) * 128],
                    rhs=aT[:, 2 * g:2 * g + 2, ts],
                    start=(g == 0), stop=False, perf_mode=DR,
                )
            nc.tensor.matmul(ps[:, j],
                             lhsT=bo_row[:, co * 128:(co + 1) * 128],
                             rhs=ones_n, start=False, stop=False)
            nc.tensor.matmul(ps[:, j], lhsT=ident, rhs=xTbf[:, co, ts],
                             start=False, stop=True)
        hh = hpool.tile([128, 2, 512], F32, tag="h")
        if eng_s:
            nc.scalar.activation(hh, ps, AF.Copy)
        else:
            nc.vector.tensor_copy(hh, ps)
        for j in range(2):
            co = 2 * cp + j
            if tail:
                eng = nc.sync if j == 0 else nc.scalar
            else:
                eng = nc.gpsimd if b == 0 else nc.sync
            eng.dma_start(out_d[b, co * 128:(co + 1) * 128, ts], hh[:, j])

    # ---- schedule: software-pipeline the two batch elements ----
    xTbf0, xT80 = load_xbf(0)
    load_weights()
    load_wq()
    xTbf1, xT81 = load_xbf(1)

    def warm_seg(n):
        # dependency-free matmuls emitted BETWEEN dependent prologue
        # matmuls: they execute while the next real matmul waits on its
        # semaphore, so the PE stays busy and the HAM clock-gate holds 8/8.
        # Each segment gets its own tile from the rotation so a real matmul
        # never waits behind a later warm-up segment.
        warm = pbig.tile([128, 2, 512], F32, tag="mm", name="warm")
        for _ in range(n):
            nc.tensor.matmul(warm[:, 0, 0:128], lhsT=wtile[:, 0:128],
                             rhs=wtile[:, 128:256], start=True, stop=True)

    # prologue: minimum work to unlock head pair 0 of batch 0.
    g0 = gn_steps((xTbf0, xT80), dve_applies=(0, 2))
    warm_seg(80)
    for s in g0[0:5]:
        s()          # b0 stats x4 + smalls -> gs matmul
    warm_seg(8)
    g0[5]()          # smalls -> bc matmul
    warm_seg(25)
    for s in g0[6:10]:
        s()          # applies (DVE/gpsimd alternating)
    qkT0, v0 = alloc_qkv()
    stage_qk(xT80, qkT0, 0, True)
    stage_qk(xT80, qkT0, 4, True)
    stage_v(xT80, v0, 0, True)

    aT0 = big.tile([128, NCHUNK, L], FP8, tag="attnT")
    aT1 = big.tile([128, NCHUNK, L], FP8, tag="attnT")
    qkT1, v1 = alloc_qkv()

    def F(fn, *a):
        return lambda: fn(*a)

    fillers = {}

    def put(slot, fn, *a):
        fillers.setdefault(slot, []).append(F(fn, *a))

    # batch-0 remaining qkv; v tile-pairs must all land by seq idx 9
    # (unit 0 consumes v kt6,7 at idx 9). Evict engines alternate.
    put(0, stage_v, xT80, v0, 1, True)
    put(1, stage_v, xT80, v0, 2, True)
    put(2, stage_v, xT80, v0, 3, True)
    put(3, stage_qk, xT80, qkT0, 1, True)
    put(5, stage_qk, xT80, qkT0, 5, True)
    put(8, stage_qk, xT80, qkT0, 2, True)
    put(10, stage_qk, xT80, qkT0, 6, True)
    put(12, stage_qk, xT80, qkT0, 3, True)
    put(14, stage_qk, xT80, qkT0, 7, True)
    # batch-1 GN (fine-grained steps so no single slot dams the DVE) + qkv
    # b1 GN: stats on ScalarE (accum_out) during unit-0's DVE-exp slots,
    # applies on gpsimd -- the DVE never touches b1's GroupNorm.
    g1 = gn_steps((xTbf1, xT81), dve_applies=(), stats_scalar=True)
    for i in range(4):
        put(1 + 2 * i, g1[i])
    put(9, g1[4])
    put(10, g1[5])
    for i in range(4):
        put(11 + i, g1[6 + i])
    put(26, stage_qk, xT81, qkT1, 0, True)
    put(28, stage_qk, xT81, qkT1, 4, True)
    put(30, stage_qk, xT81, qkT1, 1, True)
    put(33, stage_qk, xT81, qkT1, 5, True)
    # batch-0 projections th0 after units 0-3 (slot 31+LAG) finalize
    put(38, proj_cp, 0, aT0, xTbf0, 0, 0, False)
    put(38, stage_qk, xT81, qkT1, 2, True)
    put(40, proj_cp, 0, aT0, xTbf0, 1, 0, False)
    put(42, stage_qk, xT81, qkT1, 6, True)
    put(44, stage_qk, xT81, qkT1, 3, True)
    put(46, stage_qk, xT81, qkT1, 7, True)
    put(48, stage_v, xT81, v1, 0, True)
    put(51, stage_v, xT81, v1, 1, True)
    put(54, stage_v, xT81, v1, 2, True)
    put(57, stage_v, xT81, v1, 3, True)
    # batch-0 th1 projections after units 4-7 (slot 63+LAG) finalize
    put(70, proj_cp, 0, aT0, xTbf0, 0, 1, False)
    put(74, proj_cp, 0, aT0, xTbf0, 1, 1, False)
    # batch-1 th0 projections after units 8-11 (slot 95+LAG) finalize
    put(102, proj_cp, 1, aT1, xTbf1, 0, 0, False)
    put(107, proj_cp, 1, aT1, xTbf1, 1, 0, False)

    units = [(0, 0, 0), (0, 1, 0), (0, 2, 0), (0, 3, 0),
             (0, 0, 1), (0, 1, 1), (0, 2, 1), (0, 3, 1),
             (1, 0, 0), (1, 1, 0), (1, 2, 0), (1, 3, 0),
             (1, 0, 1), (1, 1, 1), (1, 2, 1), (1, 3, 1)]
    ems = [UnitEmitter(qkT0 if b == 0 else qkT1, v0 if b == 0 else v1,
                       aT0 if b == 0 else aT1, b, hp, qb,
                       sexp_kts=SEXP_KTS)
           for ui, (b, hp, qb) in enumerate(units)]
    seq = [(ui, kt) for ui in range(len(ems)) for kt in range(NTT)]
    LAG = 4
    for idx, (ui, kt) in enumerate(seq):
        ems[ui].sc_exp(kt)
        if idx >= LAG:
            uj, kj = seq[idx - LAG]
            if kj % 2 == 1:
                ems[uj].av_den((kj - 1) // 2)
            if kj == NTT - 1:
                ems[uj].finalize_a()
        if idx >= LAG + 2:
            uj2, kj2 = seq[idx - LAG - 2]
            if kj2 == NTT - 1:
                ems[uj2].finalize_b()
        for f in fillers.get(idx, ()):
            f()
    for i in range(LAG + 2):
        idx = len(seq) + i
        if idx - LAG < len(seq):
            uj, kj = seq[idx - LAG]
            if kj % 2 == 1:
                ems[uj].av_den((kj - 1) // 2)
            if kj == NTT - 1:
                ems[uj].finalize_a()
        if idx - LAG - 2 < len(seq):
            uj2, kj2 = seq[idx - LAG - 2]
            if kj2 == NTT - 1:
                ems[uj2].finalize_b()
    # tail: b1 th1 projections
    proj_cp(1, aT1, xTbf1, 0, 1, True, tail=True)
    proj_cp(1, aT1, xTbf1, 1, 1, True, tail=True)


_NC_CACHE = None


def _get_nc():
    global _NC_CACHE
    if _NC_CACHE is None:
        from contextlib import ExitStack

        nc = bacc.Bacc("TRN2", target_bir_lowering=False, debug=False)
        with tile.TileContext(nc) as tc, ExitStack() as ctx:
            build_attention_block(tc, ctx)
        nc.compile()
        _NC_CACHE = nc
    return _NC_CACHE


def _to_fp8_bytes(a):
    import ml_dtypes
    # TRN FP8_EXP4 (bias 7) matches OCP e4m3fn encodings for |x| <= 240;
    # label the bytes as float8_e4m3 (what mybir.dt.np(float8e4) maps to)
    # so the PJRT boundary does a raw byte copy, not a value conversion.
    b = np.asarray(a, np.float32).astype(ml_dtypes.float8_e4m3fn)
    return b.view(ml_dtypes.float8_e4m3)


def run(inputs, trace=False, tmpdir=None):
    """Run on 8 NeuronCores. Returns (full_output, BassKernelResults)."""
    from concourse import bass_utils
    import ml_dtypes

    x = np.asarray(inputs["x"], dtype=np.float32)
    B, H, W, Cc = x.shape
    xs = x.reshape(B, H * W, Cc).transpose(0, 2, 1)  # host pre-transpose -> [B, C, L]
    bvo = np.stack([
        np.asarray(inputs["b_qkv"], np.float32)[2 * Cc:3 * Cc],
        np.asarray(inputs["b_out"], np.float32),
    ]).astype(ml_dtypes.bfloat16)
    common = {
        "gamma": np.ascontiguousarray(np.asarray(inputs["gamma"], np.float32)),
        "beta": np.ascontiguousarray(np.asarray(inputs["beta"], np.float32)),
        "w_qkv": np.ascontiguousarray(_to_fp8_bytes(inputs["w_qkv"])),
        "b_qkv": np.ascontiguousarray(np.asarray(inputs["b_qkv"], np.float32)),
        "w_out": np.ascontiguousarray(_to_fp8_bytes(inputs["w_out"])),
        "bvo_bf": np.ascontiguousarray(bvo),
        "ident": np.ascontiguousarray(np.eye(128, dtype=ml_dtypes.bfloat16)),
    }
    n_cores = 8
    per = B // n_cores
    in_maps = [
        {"x_bf": np.ascontiguousarray(
            xs[c * per:(c + 1) * per].astype(ml_dtypes.bfloat16)),
         **common}
        for c in range(n_cores)
    ]
    nc = _get_nc()
    res = bass_utils.run_bass_kernel_spmd(
        nc, in_maps, core_ids=list(range(n_cores)), trace=trace, tmpdir=tmpdir)
    out = np.concatenate([r["out"] for r in res.results], axis=0)
    out = out.transpose(0, 2, 1)  # undo the [C, L] device layout
    return np.ascontiguousarray(out).reshape(B, H, W, Cc), res


def kernel(**inputs):
    out, _ = run(inputs, trace=False)
    return out


# revision 41
# speedup vs baseline: 1.0474x; 1.0474x over previous
"""Trainium2 Bass kernel for nn_AttentionBlock (GroupNorm + MHA + out-proj + residual).

Sharding: pure data-parallel over batch B=16 across 8 NeuronCores (2 per core).

v2 redesign vs the 205us baseline (trace-driven):
  - The kernel is elementwise-bound: exp (131K elem/partition) + PSUM
    evictions can only run on ScalarE/VectorE (~1 elem/ns each). So the
    exp stream and ALL evictions are POOLED across both engines:
    ScalarE does true exp (activation, scale/bias fused) for ~half the
    kt slots, DVE does Schraudolph bit-trick exp for the rest; qk/v/proj
    evictions go to whichever engine the slot pattern assigns.
  - qk eviction+bias on ScalarE via activation(Identity, bias=AP).
  - v bias and out-proj bias via K=1 ones-row matmuls on the PE;
    residual via identity @ x_bf16 matmul accumulated into the proj
    psum. The f32 x copy is gone (halves input DMA); evictions become
    engine-agnostic plain copies.
  - GroupNorm affine apply on GpSimd (SBUF->SBUF, per-partition scalars).
  - PSUM: one shared 3x[128,2,512] pool (scores/qkv/proj rotate through
    it) + 1 bank av + 1 bank den = exactly 8 banks.
  - Prologue: dummy exp at t=0 preloads the ACT exp table; x_bf load
    fans across both HWDGE rings; denser PE warmup keeps HAM warm.
"""
import os
import sys

for _p in ("/opt/trn_rl_repo",):
    if _p not in sys.path and os.path.isdir(_p):
        sys.path.insert(0, _p)

import numpy as np

import concourse.bass as bass
import concourse.bacc as bacc
import concourse.mybir as mybir
import concourse.tile as tile

F32 = mybir.dt.float32
BF16 = mybir.dt.bfloat16
FP8 = mybir.dt.float8e4
I16 = mybir.dt.int16
I32 = mybir.dt.int32
U8 = mybir.dt.uint8
FP8V = mybir.dt.float8e5   # wide-range fp8 for softmax weights / v

B_LOCAL = 2        # batch elements per core
L = 1024           # tokens (H*W)
C = 512            # channels
NH = 8             # heads
D = 64             # head dim
GROUPS = 32
GSIZE = C // GROUPS  # 16
EPS = 1e-5
NCHUNK = C // 128    # 4 channel chunks
NTT = L // 128       # 8 token tiles
SCALE = 1.0 / 8.0    # (1/sqrt(sqrt(64)))**2 applied inside exp
EXP_BIAS = -0.7      # common exp shift; cancels in softmax
# DVE bit-trick exp: bf16 bits of exp(SCALE*s + EXP_BIAS) ~= EXPA*s + EXPB
EXPA = 128.0 * np.log2(np.e) * SCALE
EXPB = 128.0 * (127.0 + EXP_BIAS * np.log2(np.e))
# fp8e5m2 bit-trick variant (4 bits/octave, bias 15), uint8-saturating at 0.
# e5m2's 30-octave range covers the full softmax weight spread (scores hit
# +-68 in this data; e4m3 overflows at exp>448).
EXPA8 = 4.0 * np.log2(np.e) * SCALE
EXPB8 = 4.0 * (15.0 + EXP_BIAS * np.log2(np.e))

# Scalar-engine exp slots per unit (rest go to DVE Schraudolph).
SEXP_KTS = (0, 2, 4, 6)


def build_attention_block(tc, ctx):
    nc = tc.nc
    AF = mybir.ActivationFunctionType
    OP = mybir.AluOpType
    DR = mybir.MatmulPerfMode.DoubleRow

    xbf_d = nc.dram_tensor("x_bf", [B_LOCAL, C, L], BF16, kind="ExternalInput").ap()
    gamma_d = nc.dram_tensor("gamma", [C], F32, kind="ExternalInput").ap()
    beta_d = nc.dram_tensor("beta", [C], F32, kind="ExternalInput").ap()
    wq_d = nc.dram_tensor("w_qkv", [C, 3 * C], FP8, kind="ExternalInput").ap()
    bq_d = nc.dram_tensor("b_qkv", [3 * C], F32, kind="ExternalInput").ap()
    wo_d = nc.dram_tensor("w_out", [C, C], FP8, kind="ExternalInput").ap()
    bvo_d = nc.dram_tensor("bvo_bf", [2, C], BF16, kind="ExternalInput").ap()
    ident_d = nc.dram_tensor("ident", [128, 128], BF16, kind="ExternalInput").ap()
    out_d = nc.dram_tensor("out", [B_LOCAL, C, L], F32, kind="ExternalOutput").ap()

    singles = ctx.enter_context(tc.tile_pool(name="singles", bufs=1))
    big = ctx.enter_context(tc.tile_pool(name="big", bufs=2))
    small = ctx.enter_context(tc.tile_pool(name="small", bufs=3))
    epool = ctx.enter_context(tc.tile_pool(name="epool", bufs=2))
    rpool = ctx.enter_context(tc.tile_pool(name="rpool", bufs=3))
    hpool = ctx.enter_context(tc.tile_pool(name="hpool", bufs=3))
    pbig = ctx.enter_context(tc.tile_pool(name="pbig", bufs=3, space="PSUM"))
    pav = ctx.enter_context(tc.tile_pool(name="pav", bufs=1, space="PSUM"))
    pden = ctx.enter_context(tc.tile_pool(name="pden", bufs=1, space="PSUM"))

    # ---- one-time constants ----
    # warm-up operand first: DVE memset completes in ~200ns so the PE can
    # start its warm-up spin immediately (gpsimd issues far too slowly).
    wtile = singles.tile([128, 256], BF16)
    nc.vector.memset(wtile, 0.001)
    ebias_sb = singles.tile([128, 1], F32)
    nc.gpsimd.memset(ebias_sb, EXP_BIAS)
    # dummy exp: forces the ACT exp-table load at t~0 (hidden under x DMA)
    dummy = singles.tile([128, 1], BF16)
    nc.scalar.activation(dummy, ebias_sb, AF.Exp)


    ones_sb = singles.tile([128, D], BF16)
    nc.gpsimd.memset(ones_sb, 1.0)
    ones_n = singles.tile([1, 512], BF16)
    nc.gpsimd.memset(ones_n, 1.0)

    # e_mat[c, g] = 1 iff c//16 == g (band built via two affine selects)
    e_mat = singles.tile([128, 8], F32)       # channel -> group indicator
    nc.gpsimd.memset(e_mat, 1.0)
    nc.gpsimd.affine_select(out=e_mat, in_=e_mat, compare_op=OP.is_ge,
                            fill=0.0, base=0, pattern=[[-GSIZE, 8]],
                            channel_multiplier=1)
    nc.gpsimd.affine_select(out=e_mat, in_=e_mat, compare_op=OP.is_ge,
                            fill=0.0, base=GSIZE - 1, pattern=[[GSIZE, 8]],
                            channel_multiplier=-1)
    e2_mat = singles.tile([8, 128], F32)      # group -> channel indicator
    nc.gpsimd.memset(e2_mat, 1.0)
    nc.gpsimd.affine_select(out=e2_mat, in_=e2_mat, compare_op=OP.is_ge,
                            fill=0.0, base=0, pattern=[[1, 128]],
                            channel_multiplier=-GSIZE)
    nc.gpsimd.affine_select(out=e2_mat, in_=e2_mat, compare_op=OP.is_ge,
                            fill=0.0, base=GSIZE - 1, pattern=[[-1, 128]],
                            channel_multiplier=GSIZE)

    wq8 = singles.tile([128, NCHUNK, 3 * C], FP8)
    wo8 = singles.tile([128, NCHUNK, C], FP8)
    ident = singles.tile([128, 128], BF16)
    bv_row = singles.tile([1, C], BF16)       # v bias as a K=1 weight row
    bo_row = singles.tile([1, C], BF16)       # out bias as a K=1 weight row
    gamma_sb = singles.tile([128, NCHUNK], F32)
    beta_sb = singles.tile([128, NCHUNK], F32)
    bqk_sb = singles.tile([128, 8], F32)      # q,k biases per [partition, fi]

    def load_weights():
        nc.gpsimd.dma_start(gamma_sb, gamma_d.rearrange("(o p) -> p o", p=128))
        nc.gpsimd.dma_start(beta_sb, beta_d.rearrange("(o p) -> p o", p=128))
        nc.gpsimd.dma_start(bqk_sb, bq_d[0:2 * C].rearrange("(o p) -> p o", p=128))
        nc.gpsimd.dma_start(bv_row, bvo_d[0:1, :])
        nc.gpsimd.dma_start(bo_row, bvo_d[1:2, :])
        nc.gpsimd.dma_start(ident, ident_d)

    def load_wq():
        # weights arrive host-cast to fp8e4; q,k columns first
        # (prologue-critical), then v, then w_out
        wq_r = wq_d.rearrange("(o p) f -> p o f", p=128)
        for kc in range(NCHUNK):
            nc.sync.dma_start(wq8[:, kc, 0:2 * C], wq_r[:, kc, 0:2 * C])
        for kc in range(NCHUNK):
            nc.sync.dma_start(wq8[:, kc, 2 * C:3 * C], wq_r[:, kc, 2 * C:3 * C])
        nc.sync.dma_start(wo8, wo_d.rearrange("(o p) f -> p o f", p=128))

    def load_xbf(b):
        """x^T in bf16 (host-cast). Fan the 8 half-chunk descriptors across
        both HWDGE rings so the prologue-critical load lands fast."""
        xTbf = big.tile([128, NCHUNK, L], BF16, tag="xTbf")
        xT8 = big.tile([128, NCHUNK, L], FP8, tag="xT8")
        for cc in range(NCHUNK):
            c0 = cc * 128
            eng0 = nc.sync if b == 0 else nc.scalar
            eng0.dma_start(xTbf[0:64, cc], xbf_d[b, c0:c0 + 64, :])
            nc.scalar.dma_start(xTbf[64:128, cc], xbf_d[b, c0 + 64:c0 + 128, :])
        return xTbf, xT8

    gn_scratch = singles.tile([128, 1024], BF16)

    def gn_steps(xTp, dve_applies=(0, 1), stats_scalar=False):
        """GroupNorm over all 4 chunks as a list of emission steps (so b1's
        GN can be sprinkled into filler slots). One gs/bc PE round-trip for
        the whole batch element. Applies split across DVE and GpSimd."""
        xTbf, xT8 = xTp
        box = {}

        def stats(cc):
            if cc == 0:
                box["mv"] = small.tile([128, 4, 2], F32, tag="mv", name="mv")
            if stats_scalar:
                # sum and sum-of-squares via ScalarE activation accumulators
                # (keeps b1's GN stats off the busy DVE entirely)
                nc.scalar.activation(gn_scratch, xTbf[:, cc, :], AF.Identity,
                                     accum_out=box["mv"][:, cc, 0:1])
                nc.scalar.activation(gn_scratch, xTbf[:, cc, :], AF.Square,
                                     accum_out=box["mv"][:, cc, 1:2])
                return
            st = small.tile([128, 2, 6], F32, tag="bnst")
            for s in range(2):
                nc.vector.bn_stats(st[:, s], xTbf[:, cc, s * 512:(s + 1) * 512])
            nc.vector.bn_aggr(box["mv"][:, cc, :], st)

        def smalls_gs():
            mv = box["mv"]
            if stats_scalar:
                # mv already holds [sum_c, sumsq_c]; the group matmul sums
                # them and the /(GSIZE*L) scaling happens in smalls_bc
                box["sq"] = mv
            else:
                sq = small.tile([128, 4, 2], F32, tag="sq")  # [m_c, E[x^2]_c]
                nc.vector.tensor_copy(sq[:, :, 0], mv[:, :, 0])
                nc.vector.tensor_tensor(sq[:, :, 1], mv[:, :, 0], mv[:, :, 0],
                                        op=OP.mult)
                nc.vector.tensor_tensor(sq[:, :, 1], sq[:, :, 1], mv[:, :, 1],
                                        op=OP.add)
                box["sq"] = sq
            gs = pbig.tile([8, 8], F32, tag="mm")         # per-group sums
            nc.tensor.matmul(gs, lhsT=e_mat,
                             rhs=box["sq"].rearrange("p a b -> p (a b)"),
                             start=True, stop=True)
            box["gs"] = gs

        def smalls_bc():
            gs = box["gs"]
            gsb = small.tile([8, 4, 2], F32, tag="gsb")
            scl = 1.0 / (GSIZE * L) if stats_scalar else 1.0 / GSIZE
            nc.vector.tensor_scalar_mul(gsb, gs.rearrange("p (a b) -> p a b", b=2),
                                        scl)              # [m_g, E[x^2]_g]
            var = small.tile([8, 4], F32, tag="var")
            nc.vector.tensor_tensor(var, gsb[:, :, 0], gsb[:, :, 0], op=OP.mult)
            nc.vector.tensor_tensor(var, gsb[:, :, 1], var, op=OP.subtract)
            nc.vector.tensor_scalar(out=var, in0=var, scalar1=float(EPS),
                                    scalar2=None, op0=OP.add)
            # rstd = rsqrt(var+eps): Quake-III seed + two Newton steps
            yi = small.tile([8, 4], I32, tag="yi")
            nc.vector.tensor_scalar(out=yi, in0=var.bitcast(I32),
                                    scalar1=1, scalar2=None,
                                    op0=OP.arith_shift_right)
            nc.vector.tensor_scalar(out=yi, in0=yi, scalar1=-1, scalar2=0x5F3759DF,
                                    op0=OP.mult, op1=OP.add)
            y = yi.bitcast(F32)
            t = small.tile([8, 4], F32, tag="nrt")
            for _ in range(2):
                nc.vector.tensor_tensor(t, y, y, op=OP.mult)
                nc.vector.tensor_tensor(t, t, var, op=OP.mult)
                nc.vector.tensor_scalar(out=t, in0=t, scalar1=-0.5, scalar2=1.5,
                                        op0=OP.mult, op1=OP.add)
                nc.vector.tensor_tensor(y, y, t, op=OP.mult)
            nc.vector.tensor_copy(gsb[:, :, 1], y)        # gsb = [m_g, rstd_g]
            bc = pbig.tile([128, 8], F32, tag="mm")       # broadcast via PE
            nc.tensor.matmul(bc, lhsT=e2_mat,
                             rhs=gsb.rearrange("p a b -> p (a b)"),
                             start=True, stop=True)
            bc2 = bc.rearrange("p (a b) -> p a b", b=2)
            ab = small.tile([128, 4, 2], F32, tag="ab")
            nc.vector.tensor_tensor(ab[:, :, 0], bc2[:, :, 1], gamma_sb,
                                    op=OP.mult)
            nc.vector.tensor_tensor(ab[:, :, 1], bc2[:, :, 0], ab[:, :, 0],
                                    op=OP.mult)
            nc.vector.tensor_tensor(ab[:, :, 1], beta_sb, ab[:, :, 1],
                                    op=OP.subtract)
            box["ab"] = ab

        def apply(cc):
            ab = box["ab"]
            eng = nc.vector if cc in dve_applies else nc.gpsimd
            eng.tensor_scalar(out=xT8[:, cc, :], in0=xTbf[:, cc, :],
                              scalar1=ab[:, cc, 0:1], scalar2=ab[:, cc, 1:2],
                              op0=OP.mult, op1=OP.add)

        return [lambda: stats(0), lambda: stats(1), lambda: stats(2),
                lambda: stats(3), smalls_gs, smalls_bc,
                lambda: apply(0), lambda: apply(1), lambda: apply(2),
                lambda: apply(3)]

    def alloc_qkv():
        qkT = big.tile([128, 8, L], BF16, tag="qkT")
        v_sb = big.tile([128, NTT, NH, D], BF16, tag="v")
        return qkT, v_sb

    def stage_qk(xT8, qkT, fi, eng_s):
        """One q/k feature tile fi (128 features x full L) : 4 DR matmuls into
        a [128,2,512] psum, one pooled evict+bias."""
        ps = pbig.tile([128, 2, 512], F32, tag="mm")
        for tb in range(2):
            for g in range(2):
                nc.tensor.matmul(
                    ps[:, tb],
                    lhsT=wq8[:, 2 * g:2 * g + 2, fi * 128:(fi + 1) * 128],
                    rhs=xT8[:, 2 * g:2 * g + 2, tb * 512:(tb + 1) * 512],
                    start=(g == 0), stop=(g == 1), perf_mode=DR,
                )
        if eng_s:
            nc.scalar.activation(qkT[:, fi, :].rearrange("p (a b) -> p a b", a=2),
                                 ps, AF.Identity, bias=bqk_sb[:, fi:fi + 1],
                                 scale=1.0)
        else:
            nc.vector.tensor_scalar(
                out=qkT[:, fi, :].rearrange("p (a b) -> p a b", a=2), in0=ps,
                scalar1=bqk_sb[:, fi:fi + 1], scalar2=None, op0=OP.add)

    def stage_v(xT8, v_sb, tp, eng_s):
        """v for token-tile pair (2tp, 2tp+1): fp8 DR matmuls + K=1 ones-row
        bias matmul, then an engine-agnostic copy eviction."""
        ps = pbig.tile([128, 2, 512], F32, tag="mm")
        for half in range(2):
            tt = 2 * tp + half
            for g in range(2):
                nc.tensor.matmul(
                    ps[:, half],
                    lhsT=xT8[:, 2 * g:2 * g + 2, tt * 128:(tt + 1) * 128],
                    rhs=wq8[:, 2 * g:2 * g + 2, 2 * C:3 * C],
                    start=(g == 0), stop=False, perf_mode=DR,
                )
            nc.tensor.matmul(ps[:, half], lhsT=ones_n[:, 0:128], rhs=bv_row,
                             start=False, stop=True)
        dst = v_sb[:, 2 * tp:2 * tp + 2].rearrange("p a h d -> p a (h d)")
        if eng_s:
            nc.scalar.activation(dst, ps, AF.Copy)
        else:
            nc.vector.tensor_copy(dst, ps)

    class UnitEmitter:
        """Attention for head pair (2*hp, 2*hp+1), query half qb.
        Per k-tile: row-packed score pair -> one exp on ScalarE (true exp)
        or DVE (Schraudolph bit-trick) -> col-packed attn@v pair + den."""

        def __init__(self, qkT, v_sb, aT, b, hp, qb, sexp_kts=SEXP_KTS):
            self.qkT, self.v_sb, self.aT = qkT, v_sb, aT
            self.b, self.hp, self.qb = b, hp, qb
            self.qs = slice(qb * 512, (qb + 1) * 512)
            self.sexp_kts = sexp_kts
            self.ready = False

        def sc_exp(self, kt):
            if not self.ready:
                self.ebf = epool.tile([128, NTT, 2, 512], BF16, tag="e")
                self.ready = True
            hp, qs = self.hp, self.qs
            ks = slice(kt * 128, (kt + 1) * 128)
            sc = pbig.tile([128, 2, 512], F32, tag="mm")
            nc.tensor.matmul(sc[:, 0], lhsT=self.qkT[0:64, 4 + hp, ks],
                             rhs=self.qkT[0:64, hp, qs], start=True, stop=True)
            nc.tensor.matmul(sc[:, 1], lhsT=self.qkT[64:128, 4 + hp, ks],
                             rhs=self.qkT[64:128, hp, qs], start=True, stop=True)
            if kt in self.sexp_kts:
                nc.scalar.activation(self.ebf[:, kt], sc, AF.Exp, bias=ebias_sb,
                                     scale=SCALE)
            else:
                # Schraudolph exp on DVE: write the bf16 bit pattern of
                # exp(SCALE*s + EXP_BIAS) via one int16 linear op
                nc.vector.tensor_scalar(
                    out=self.ebf[:, kt].bitcast(I16), in0=sc,
                    scalar1=float(EXPA), scalar2=float(EXPB),
                    op0=OP.mult, op1=OP.add)

        def av_den(self, g):
            if g == 0:
                self.out_p = pav.tile([128, 512], F32, tag="aout")
                self.den_p = pden.tile([128, 512], F32, tag="aden")
            h0, h1 = 2 * self.hp, 2 * self.hp + 1
            for kt in (2 * g, 2 * g + 1):
                nc.tensor.matmul(self.out_p[0:64, :], lhsT=self.v_sb[:, kt, h0],
                                 rhs=self.ebf[:, kt, 0], start=(kt == 0),
                                 stop=(kt == NTT - 1), skip_group_check=True)
                nc.tensor.matmul(self.out_p[64:128, :], lhsT=self.v_sb[:, kt, h1],
                                 rhs=self.ebf[:, kt, 1], start=(kt == 0),
                                 stop=(kt == NTT - 1), skip_group_check=True)
                nc.tensor.matmul(self.den_p[0:64, :], lhsT=ones_sb,
                                 rhs=self.ebf[:, kt, 0], start=(kt == 0),
                                 stop=(kt == NTT - 1), skip_group_check=True)
                nc.tensor.matmul(self.den_p[64:128, :], lhsT=ones_sb,
                                 rhs=self.ebf[:, kt, 1], start=(kt == 0),
                                 stop=(kt == NTT - 1), skip_group_check=True)

        def finalize_a(self):
            rc = rpool.tile([128, 512], F32, tag="rc")
            nc.vector.reciprocal_approx_fast(rc, self.den_p)
            nc.vector.tensor_tensor(out=self.aT[:, self.hp, self.qs],
                                    in0=self.out_p, in1=rc, op=OP.mult)

        def finalize_b(self):
            pass

    def proj_cp(b, aT, xTbf, cp, th, eng_s, tail=False):
        """Transposed out-projection for channel chunks (2cp, 2cp+1), token
        half th: fp8 DR matmuls + identity@x residual + K=1 b_out row,
        then a pooled copy eviction and the store."""
        ts = slice(th * 512, (th + 1) * 512)
        ps = pbig.tile([128, 2, 512], F32, tag="mm")
        for j in range(2):
            co = 2 * cp + j
            for g in range(2):
                nc.tensor.matmul(
                    ps[:, j],
                    lhsT=wo8[:, 2 * g:2 * g + 2, co * 128:(co + 1) * 128],
                    rhs=aT[:, 2 * g:2 * g + 2, ts],
                    start=(g == 0), stop=False, perf_mode=DR,
                )
            nc.tensor.matmul(ps[:, j],
                             lhsT=bo_row[:, co * 128:(co + 1) * 128],
                             rhs=ones_n, start=False, stop=False)
            nc.tensor.matmul(ps[:, j], lhsT=ident, rhs=xTbf[:, co, ts],
                             start=False, stop=True)
        hh = hpool.tile([128, 2, 512], F32, tag="h")
        if eng_s:
            nc.scalar.activation(hh, ps, AF.Copy)
        else:
            nc.vector.tensor_copy(hh, ps)
        for j in range(2):
            co = 2 * cp + j
            if tail:
                eng = nc.sync if j == 0 else nc.scalar
            else:
                eng = nc.gpsimd if b == 0 else nc.sync
            eng.dma_start(out_d[b, co * 128:(co + 1) * 128, ts], hh[:, j])

    # ---- schedule: software-pipeline the two batch elements ----
    xTbf0, xT80 = load_xbf(0)
    load_weights()
    load_wq()
    xTbf1, xT81 = load_xbf(1)

    def warm_seg(n):
        # dependency-free matmuls emitted BETWEEN dependent prologue
        # matmuls: they execute while the next real matmul waits on its
        # semaphore, so the PE stays busy and the HAM clock-gate holds 8/8.
        # Each segment gets its own tile from the rotation so a real matmul
        # never waits behind a later warm-up segment.
        warm = pbig.tile([128, 2, 512], F32, tag="mm", name="warm")
        for _ in range(n):
            nc.tensor.matmul(warm[:, 0, 0:128], lhsT=wtile[:, 0:128],
                             rhs=wtile[:, 128:256], start=True, stop=True)

    # prologue: minimum work to unlock head pair 0 of batch 0.
    g0 = gn_steps((xTbf0, xT80), dve_applies=(0, 2))
    warm_seg(80)
    for s in g0[0:5]:
        s()          # b0 stats x4 + smalls -> gs matmul
    warm_seg(8)
    g0[5]()          # smalls -> bc matmul
    warm_seg(25)
    for s in g0[6:10]:
        s()          # applies (DVE/gpsimd alternating)
    qkT0, v0 = alloc_qkv()
    stage_qk(xT80, qkT0, 0, True)
    stage_qk(xT80, qkT0, 4, True)
    stage_v(xT80, v0, 0, True)

    aT0 = big.tile([128, NCHUNK, L], FP8, tag="attnT")
    aT1 = big.tile([128, NCHUNK, L], FP8, tag="attnT")
    qkT1, v1 = alloc_qkv()

    def F(fn, *a):
        return lambda: fn(*a)

    fillers = {}

    def put(slot, fn, *a):
        fillers.setdefault(slot, []).append(F(fn, *a))

    # batch-0 remaining qkv; v tile-pairs must all land by seq idx 9
    # (unit 0 consumes v kt6,7 at idx 9). Evict engines alternate.
    put(0, stage_v, xT80, v0, 1, True)
    put(1, stage_v, xT80, v0, 2, True)
    put(2, stage_v, xT80, v0, 3, True)
    put(3, stage_qk, xT80, qkT0, 1, True)
    put(5, stage_qk, xT80, qkT0, 5, True)
    put(8, stage_qk, xT80, qkT0, 2, True)
    put(10, stage_qk, xT80, qkT0, 6, True)
    put(12, stage_qk, xT80, qkT0, 3, True)
    put(14, stage_qk, xT80, qkT0, 7, True)
    # batch-1 GN (fine-grained steps so no single slot dams the DVE) + qkv
    # b1 GN: stats on ScalarE (accum_out) during unit-0's DVE-exp slots,
    # applies on gpsimd -- the DVE never touches b1's GroupNorm.
    g1 = gn_steps((xTbf1, xT81), dve_applies=(), stats_scalar=True)
    for i in range(4):
        put(1 + 2 * i, g1[i])
    put(9, g1[4])
    put(10, g1[5])
    for i in range(4):
        put(11 + i, g1[6 + i])
    put(26, stage_qk, xT81, qkT1, 0, True)
    put(28, stage_qk, xT81, qkT1, 4, True)
    put(30, stage_qk, xT81, qkT1, 1, True)
    put(33, stage_qk, xT81, qkT1, 5, True)
    # batch-0 projections th0 after units 0-3 (slot 31+LAG) finalize
    put(38, proj_cp, 0, aT0, xTbf0, 0, 0, False)
    put(38, stage_qk, xT81, qkT1, 2, True)
    put(40, proj_cp, 0, aT0, xTbf0, 1, 0, False)
    put(42, stage_qk, xT81, qkT1, 6, True)
    put(44, stage_qk, xT81, qkT1, 3, True)
    put(46, stage_qk, xT81, qkT1, 7, True)
    put(48, stage_v, xT81, v1, 0, True)
    put(51, stage_v, xT81, v1, 1, True)
    put(54, stage_v, xT81, v1, 2, True)
    put(57, stage_v, xT81, v1, 3, True)
    # batch-0 th1 projections after units 4-7 (slot 63+LAG) finalize
    put(70, proj_cp, 0, aT0, xTbf0, 0, 1, False)
    put(74, proj_cp, 0, aT0, xTbf0, 1, 1, False)
    # batch-1 th0 projections after units 8-11 (slot 95+LAG) finalize
    put(102, proj_cp, 1, aT1, xTbf1, 0, 0, False)
    put(107, proj_cp, 1, aT1, xTbf1, 1, 0, False)

    units = [(0, 0, 0), (0, 1, 0), (0, 2, 0), (0, 3, 0),
             (0, 0, 1), (0, 1, 1), (0, 2, 1), (0, 3, 1),
             (1, 0, 0), (1, 1, 0), (1, 2, 0), (1, 3, 0),
             (1, 0, 1), (1, 1, 1), (1, 2, 1), (1, 3, 1)]
    ems = [UnitEmitter(qkT0 if b == 0 else qkT1, v0 if b == 0 else v1,
                       aT0 if b == 0 else aT1, b, hp, qb,
                       sexp_kts=SEXP_KTS)
           for ui, (b, hp, qb) in enumerate(units)]
    seq = [(ui, kt) for ui in range(len(ems)) for kt in range(NTT)]
    LAG = 6
    for idx, (ui, kt) in enumerate(seq):
        ems[ui].sc_exp(kt)
        if idx >= LAG:
            uj, kj = seq[idx - LAG]
            if kj % 2 == 1:
                ems[uj].av_den((kj - 1) // 2)
            if kj == NTT - 1:
                ems[uj].finalize_a()
        if idx >= LAG + 2:
            uj2, kj2 = seq[idx - LAG - 2]
            if kj2 == NTT - 1:
                ems[uj2].finalize_b()
        for f in fillers.get(idx, ()):
            f()
    for i in range(LAG + 2):
        idx = len(seq) + i
        if idx - LAG < len(seq):
            uj, kj = seq[idx - LAG]
            if kj % 2 == 1:
                ems[uj].av_den((kj - 1) // 2)
            if kj == NTT - 1:
                ems[uj].finalize_a()
        if idx - LAG - 2 < len(seq):
            uj2, kj2 = seq[idx - LAG - 2]
            if kj2 == NTT - 1:
                ems[uj2].finalize_b()
    # tail: b1 th1 projections
    proj_cp(1, aT1, xTbf1, 0, 1, True, tail=True)
    proj_cp(1, aT1, xTbf1, 1, 1, True, tail=True)


_NC_CACHE = None


def _get_nc():
    global _NC_CACHE
    if _NC_CACHE is None:
        from contextlib import ExitStack

        nc = bacc.Bacc("TRN2", target_bir_lowering=False, debug=False)
        with tile.TileContext(nc) as tc, ExitStack() as ctx:
            build_attention_block(tc, ctx)
        nc.compile()
        _NC_CACHE = nc
    return _NC_CACHE


def _to_fp8_bytes(a):
    import ml_dtypes
    # TRN FP8_EXP4 (bias 7) matches OCP e4m3fn encodings for |x| <= 240;
    # label the bytes as float8_e4m3 (what mybir.dt.np(float8e4) maps to)
    # so the PJRT boundary does a raw byte copy, not a value conversion.
    b = np.asarray(a, np.float32).astype(ml_dtypes.float8_e4m3fn)
    return b.view(ml_dtypes.float8_e4m3)


def run(inputs, trace=False, tmpdir=None):
    """Run on 8 NeuronCores. Returns (full_output, BassKernelResults)."""
    from concourse import bass_utils
    import ml_dtypes

    x = np.asarray(inputs["x"], dtype=np.float32)
    B, H, W, Cc = x.shape
    xs = x.reshape(B, H * W, Cc).transpose(0, 2, 1)  # host pre-transpose -> [B, C, L]
    bvo = np.stack([
        np.asarray(inputs["b_qkv"], np.float32)[2 * Cc:3 * Cc],
        np.asarray(inputs["b_out"], np.float32),
    ]).astype(ml_dtypes.bfloat16)
    common = {
        "gamma": np.ascontiguousarray(np.asarray(inputs["gamma"], np.float32)),
        "beta": np.ascontiguousarray(np.asarray(inputs["beta"], np.float32)),
        "w_qkv": np.ascontiguousarray(_to_fp8_bytes(inputs["w_qkv"])),
        "b_qkv": np.ascontiguousarray(np.asarray(inputs["b_qkv"], np.float32)),
        "w_out": np.ascontiguousarray(_to_fp8_bytes(inputs["w_out"])),
        "bvo_bf": np.ascontiguousarray(bvo),
        "ident": np.ascontiguousarray(np.eye(128, dtype=ml_dtypes.bfloat16)),
    }
    n_cores = 8
    per = B // n_cores
    in_maps = [
        {"x_bf": np.ascontiguousarray(
            xs[c * per:(c + 1) * per].astype(ml_dtypes.bfloat16)),
         **common}
        for c in range(n_cores)
    ]
    nc = _get_nc()
    res = bass_utils.run_bass_kernel_spmd(
        nc, in_maps, core_ids=list(range(n_cores)), trace=trace, tmpdir=tmpdir)
    out = np.concatenate([r["out"] for r in res.results], axis=0)
    out = out.transpose(0, 2, 1)  # undo the [C, L] device layout
    return np.ascontiguousarray(out).reshape(B, H, W, Cc), res


def kernel(**inputs):
    out, _ = run(inputs, trace=False)
    return out
